# revision 1
# baseline (speedup 1.0000x reference)
"""AQT int8 symmetric-quantized dot_general (bmk,kn->bmn) on 8 TRN2 NeuronCores.

Problem: lhs [2, 4096, 4096] f32, rhs [4096, 4096] f32.
  q_l, s_l = absmax-int8-quantize(lhs, axis=K)   (per-row scales)
  q_r, s_r = absmax-int8-quantize(rhs, axis=K)   (per-col scales)
  out = (q_l @ q_r) * s_l * s_r                  [2, 4096, 4096] f32

Sharding: 2 (batch) x 4 (N columns) grid over 8 cores; K replicated.
Each core computes an independent [4096, 1024] output block - no collectives.

Per-core kernel (Tile framework):
  - rhs pass 1: stream k-tile pairs, |x| (ACT, bf16 out) + running max
    (DVE bf16 2x mode), then gpsimd partition_all_reduce -> per-column
    amax. bf16 amax costs ~0.1% scale deviation (rel err ~5e-3 total,
    gate is 2e-2) and halves the DVE-serial startup chain.
  - rhs pass 2: re-stream k-tile pairs, q_r = round(rhs * (127/amax))
    using direct f32->int32 conversion (round-half-even, matching
    jnp.round), then int32 -> bf16 on ACT (int8 values are exact in bf16).
  - lhs per m-tile of 128 rows: free-axis amax reduce, quantize+round via
    the fp32 magic-number trick, then ONE xbar DMA-transpose instruction
    block-transposes all 32 128x128 tiles to put K on partitions, then
    32 accumulating matmuls per 512-wide output panel (bf16 x bf16 -> f32).
  - epilogue: out = (psum * s_l) * s_r fused in one DVE op, DMA out.
  - first two m-tiles are prepped before the rhs passes so the PE starts
    (and HAM-warms) during the DVE-serial rhs amax/quantize chains.
"""

import numpy as np

import concourse.bass as bass
import concourse.mybir as mybir
import concourse.tile as tile
from concourse import bacc, bass_isa
from concourse.bass import ts
from concourse.bass_utils import run_bass_kernel_spmd
MAGIC = 12582912.0  # 1.5 * 2**23: fp32 add => round-half-even to integer

B, M, K, N = 2, 4096, 4096, 4096
GRID_B, GRID_N = 2, 4  # 8 cores
M_LOC, N_LOC = M, N // GRID_N


def build_nc(m_loc=M_LOC, k=K, n_loc=N_LOC, panel=512):
    f32, bf16 = mybir.dt.float32, mybir.dt.bfloat16
    mult, add = mybir.AluOpType.mult, mybir.AluOpType.add
    nk, nm, npan = k // 128, m_loc // 128, n_loc // panel
    nc = bacc.Bacc("TRN2", target_bir_lowering=False, debug=False)
    lhs_d = nc.dram_tensor("lhs", [m_loc, k], f32, kind="ExternalInput")
    rhs_d = nc.dram_tensor("rhs", [k, n_loc], f32, kind="ExternalInput")
    out_d = nc.dram_tensor("out", [m_loc, n_loc], f32, kind="ExternalOutput")

    with tile.TileContext(nc) as tc:
        with (
            tc.tile_pool(name="const", bufs=1) as constp,
            tc.tile_pool(name="qr", bufs=1) as qrp,
            tc.tile_pool(name="rstat", bufs=1) as rstatp,
            tc.tile_pool(name="rio", bufs=3) as riop,
            tc.tile_pool(name="rtmp", bufs=2) as rtmpp,
            tc.tile_pool(name="lio", bufs=2) as liop,
            tc.tile_pool(name="lq", bufs=2) as lqp,
            tc.tile_pool(name="lstat", bufs=8) as lstatp,
            tc.tile_pool(name="eo", bufs=4) as eop,
            tc.tile_pool(name="pout", bufs=4, space="PSUM") as poutp,
        ):
            # lhs m-tile prep: quantize + xbar-transpose -> (qT, s_l)
            def prep_mtile(mi):
                lt = liop.tile([128, k], f32, tag="lt")
                nc.sync.dma_start(lt[:], lhs_d[ts(mi, 128), :])
                am = lstatp.tile([128, 1], f32, tag="am")
                nc.vector.tensor_reduce(
                    am[:],
                    lt[:],
                    axis=mybir.AxisListType.X,
                    op=mybir.AluOpType.max,
                    apply_absolute_value=True,
                )
                inv_l = lstatp.tile([128, 1], f32, tag="invl")
                nc.vector.reciprocal(inv_l[:], am[:])
                nc.vector.tensor_scalar_mul(inv_l[:], inv_l[:], 127.0)
                s_l = lstatp.tile([128, 1], f32, tag="sl")
                nc.vector.tensor_scalar_mul(s_l[:], am[:], 1.0 / 127.0)
                # in-place: lt = lt * inv_l + MAGIC  (rounds to int at the add)
                nc.vector.tensor_scalar(
                    lt[:], lt[:], inv_l[:], MAGIC, op0=mult, op1=add
                )
                qb = lqp.tile([128, k], bf16, tag="qb")
                nc.scalar.activation(
                    qb[:], lt[:], mybir.ActivationFunctionType.Copy, bias=-MAGIC
                )
                qT = lqp.tile([128, k], bf16, tag="qT")
                # one xbar-transpose DMA does all nk 128x128 block transposes:
                # out[p, b, f] = qb[f, b*128 + p]
                nc.sync.dma_start_transpose(
                    qT[:].rearrange("p (b f) -> p b f", f=128), qb[:]
                )
                return qT, s_l

            def mm_mtile(mi, qT, s_l):
                for p in range(npan):
                    po = poutp.tile([128, panel], f32, tag="po")
                    for kk in range(nk):
                        nc.tensor.matmul(
                            po[:],
                            qT[:, ts(kk, 128)],
                            qr_tiles[kk][:, ts(p, panel)],
                            start=(kk == 0),
                            stop=(kk == nk - 1),
                        )
                    eo = eop.tile([128, panel], f32, tag="eo")
                    nc.vector.scalar_tensor_tensor(
                        eo[:], po[:], s_l[:], s_r[:, ts(p, panel)], op0=mult, op1=mult
                    )
                    nc.scalar.dma_start(out_d[ts(mi, 128), ts(p, panel)], eo[:])

            # prep the first lhs tiles BEFORE rhs passes: their DVE/ACT work and
            # the first matmuls/transposes run during the (DVE-serial) rhs amax chain
            prepped = {}
            n_pre = min(2, nm)
            for mi in range(n_pre):
                prepped[mi] = prep_mtile(mi)

            # ---- rhs pass 1: per-column amax (exact f32) ----
            # batch 2 k-tiles per op: DMA [128, 2, n_loc], reduce op overheads
            acc = rstatp.tile([128, 2 * n_loc], bf16, tag="acc")
            nc.vector.memset(acc[:], 0.0)
            for kk in range(0, nk, 2):
                rt = riop.tile([128, 2 * n_loc], f32, tag="rt")
                nc.sync.dma_start(
                    rt[:].rearrange("p (t n) -> p t n", t=2),
                    rhs_d[ts(kk // 2, 256), :].rearrange("(t p) n -> p t n", p=128),
                )
                ra = rtmpp.tile([128, 2 * n_loc], bf16, tag="rab")
                nc.scalar.activation(ra[:], rt[:], mybir.ActivationFunctionType.Abs)
                nc.vector.tensor_tensor(
                    acc[:], acc[:], ra[:], op=mybir.AluOpType.max
                )
            accm = rtmpp.tile([128, n_loc], f32, tag="ra")
            nc.vector.tensor_tensor(
                accm[:], acc[:, 0:n_loc], acc[:, n_loc : 2 * n_loc],
                op=mybir.AluOpType.max,
            )
            amax_r = rstatp.tile([128, n_loc], f32, tag="amax_r")
            nc.gpsimd.partition_all_reduce(
                amax_r[:], accm[:], channels=128, reduce_op=bass_isa.ReduceOp.absmax
            )
            inv_r = rstatp.tile([128, n_loc], f32, tag="inv_r")
            nc.vector.reciprocal_approx_fast(inv_r[:], amax_r[:])
            nc.vector.tensor_scalar_mul(inv_r[:], inv_r[:], 127.0)
            inv_r2 = (
                inv_r[:]
                .rearrange("p (o n) -> p o n", o=1)
                .broadcast_to((128, 2, n_loc))
            )
            s_r = rtmpp.tile([128, n_loc], f32, tag="ra")
            nc.vector.tensor_scalar_mul(s_r[:], amax_r[:], 1.0 / 127.0)

            # ---- rhs pass 2: quantize via direct f32->int32 (round-half-even)
            qr_tiles = []
            for kk in range(0, nk, 2):
                rt = riop.tile([128, 2 * n_loc], f32, tag="rt")
                nc.sync.dma_start(
                    rt[:].rearrange("p (t n) -> p t n", t=2),
                    rhs_d[ts(kk // 2, 256), :].rearrange("(t p) n -> p t n", p=128),
                )
                ru = rtmpp.tile([128, 2 * n_loc], mybir.dt.int32, tag="ru")
                nc.vector.tensor_tensor(
                    ru[:].rearrange("p (o n) -> p o n", o=2),
                    rt[:].rearrange("p (o n) -> p o n", o=2),
                    inv_r2,
                    op=mult,
                )
                for t in range(2):
                    qr = qrp.tile([128, n_loc], bf16, tag=f"qr{kk + t}")
                    nc.scalar.copy(qr[:], ru[:, t * n_loc : (t + 1) * n_loc])
                    qr_tiles.append(qr)

            # ---- m-tile loop: matmuls + epilogue, prepping ahead ----
            for mi in range(nm):
                if mi not in prepped:
                    prepped[mi] = prep_mtile(mi)
                qT, s_l = prepped.pop(mi)
                mm_mtile(mi, qT, s_l)
                nxt = mi + n_pre
                if nxt < nm and nxt not in prepped:
                    prepped[nxt] = prep_mtile(nxt)

    nc.compile()
    return nc


def run_shards(nc, lhs_shards, rhs_shards, trace=False, **kw):
    in_maps = [
        {"lhs": np.ascontiguousarray(l), "rhs": np.ascontiguousarray(r)}
        for l, r in zip(lhs_shards, rhs_shards)
    ]
    return run_bass_kernel_spmd(
        nc, in_maps, core_ids=list(range(len(in_maps))), trace=trace, **kw
    )


_NC_CACHE = {}


def get_full_nc():
    if "nc" not in _NC_CACHE:
        _NC_CACHE["nc"] = build_nc()
    return _NC_CACHE["nc"]


def kernel(lhs, rhs):
    lhs = np.ascontiguousarray(np.asarray(lhs, dtype=np.float32))
    rhs = np.ascontiguousarray(np.asarray(rhs, dtype=np.float32))
    assert lhs.shape == (B, M, K) and rhs.shape == (K, N)
    nc = get_full_nc()
    lhs_shards, rhs_shards = [], []
    for c in range(8):
        pi, qi = c // GRID_N, c % GRID_N
        lhs_shards.append(lhs[pi])
        rhs_shards.append(rhs[:, qi * N_LOC : (qi + 1) * N_LOC])
    res = run_shards(nc, lhs_shards, rhs_shards)
    out = np.empty((B, M, N), np.float32)
    for c in range(8):
        pi, qi = c // GRID_N, c % GRID_N
        out[pi, :, qi * N_LOC : (qi + 1) * N_LOC] = res.results[c]["out"]
    return out


if __name__ == "__main__":
    rng = np.random.default_rng(0)
    lhs = rng.standard_normal((B, M, K), dtype=np.float32)
    rhs = rng.standard_normal((K, N), dtype=np.float32)
    out = kernel(lhs=lhs, rhs=rhs)
    print("kernel output:", out.shape, out.dtype)



# revision 3
# speedup vs baseline: 1.0327x; 1.0327x over previous
"""AQT int8 symmetric-quantized dot_general (bmk,kn->bmn) on 8 TRN2 NeuronCores.

Problem: lhs [2, 4096, 4096] f32, rhs [4096, 4096] f32.
  q_l, s_l = absmax-int8-quantize(lhs, axis=K)   (per-row scales)
  q_r, s_r = absmax-int8-quantize(rhs, axis=K)   (per-col scales)
  out = (q_l @ q_r) * s_l * s_r                  [2, 4096, 4096] f32

Sharding: 2 (batch) x 4 (N columns) grid over 8 cores; K replicated.
Each core computes an independent [4096, 1024] output block - no collectives.

Per-core kernel (Tile framework), v2:
  - rhs single HBM pass: stream 16 groups of [128, 2x1024] f32; scalar engine
    makes a persistent SIGNED bf16 copy (sb); DVE keeps running max and min
    accumulators (bf16, 2x mode) - no ABS pass, no HBM re-read.
  - amax = max(maxacc, -minacc) folded in 2 DVE ops, gpsimd
    partition_all_reduce -> per-column amax; inv = 127*recip(amax) in bf16.
  - rhs quantize from SBUF: ru_int16 = rne(sb * inv) (DVE 2x), copy back
    into sb as bf16 (DVE 2x) - sb becomes q_r in place; no qr pool.
  - lhs per m-tile of 128 rows: DVE amax reduce; the quantize multiply runs
    on the SCALAR engine: act(lt*inv_l + MAGIC) in-place f32, then
    act(lt - MAGIC) -> bf16; one xbar DMA-transpose puts K on partitions;
    32 accumulating matmuls per 512-wide output panel (bf16 -> f32 psum).
  - epilogue: out = (psum * s_l) * s_r fused in one DVE op, DMA out f32.
  - prep runs 3 m-tiles ahead so the PE never waits at tile boundaries
    (gaps reset the PE p-state ramp, costing 2x cycle time for ~3us).
"""

import numpy as np

import concourse.bass as bass
import concourse.mybir as mybir
import concourse.tile as tile
from concourse import bacc, bass_isa
from concourse.bass import ts
from concourse.bass_utils import run_bass_kernel_spmd

MAGIC = 12582912.0  # 1.5 * 2**23: fp32 add => round-half-even to integer

B, M, K, N = 2, 4096, 4096, 4096
GRID_B, GRID_N = 2, 4  # 8 cores
M_LOC, N_LOC = M, N // GRID_N


def build_nc(m_loc=M_LOC, k=K, n_loc=N_LOC, panel=512):
    f32, bf16, i16 = mybir.dt.float32, mybir.dt.bfloat16, mybir.dt.int16
    mult, add = mybir.AluOpType.mult, mybir.AluOpType.add
    vmax, vmin = mybir.AluOpType.max, mybir.AluOpType.min
    nk, nm, npan = k // 128, m_loc // 128, n_loc // panel
    ng = nk // 2  # rhs DMA groups of 2 k-tiles
    nc = bacc.Bacc("TRN2", target_bir_lowering=False, debug=False)
    lhs_d = nc.dram_tensor("lhs", [m_loc, k], f32, kind="ExternalInput")
    rhs_d = nc.dram_tensor("rhs", [k, n_loc], f32, kind="ExternalInput")
    out_d = nc.dram_tensor("out", [m_loc, n_loc], f32, kind="ExternalOutput")

    with tile.TileContext(nc) as tc:
        with (
            tc.tile_pool(name="rstat", bufs=1) as rstatp,
            tc.tile_pool(name="rio", bufs=3) as riop,
            tc.tile_pool(name="sb", bufs=1) as sbp,
            tc.tile_pool(name="rtmp", bufs=2) as rtmpp,
            tc.tile_pool(name="lio", bufs=2) as liop,
            tc.tile_pool(name="lqb", bufs=2) as lqbp,
            tc.tile_pool(name="lqt", bufs=3) as lqtp,
            tc.tile_pool(name="lstat", bufs=8) as lstatp,
            tc.tile_pool(name="eo", bufs=4) as eop,
            tc.tile_pool(name="pout", bufs=4, space="PSUM") as poutp,
        ):
            # ---------- rhs pass 1: stream + signed bf16 copy + max/min ----
            accA = rstatp.tile([128, 2 * n_loc], bf16, tag="accA")
            accB = rstatp.tile([128, 2 * n_loc], bf16, tag="accB")
            nc.gpsimd.memset(accA[:], 0.0)
            nc.gpsimd.memset(accB[:], 0.0)

            sb_tiles = []

            def rhs_group(g):
                rt = riop.tile([128, 2 * n_loc], f32, tag="rt")
                nc.sync.dma_start(
                    rt[:].rearrange("p (t n) -> p t n", t=2),
                    rhs_d[ts(g, 256), :].rearrange("(t p) n -> p t n", p=128),
                )
                sb = sbp.tile([128, 2 * n_loc], bf16, tag=f"sb{g}")
                nc.scalar.copy(sb[:], rt[:])
                nc.vector.tensor_tensor(accA[:], accA[:], sb[:], op=vmax)
                nc.vector.tensor_tensor(accB[:], accB[:], sb[:], op=vmin)
                sb_tiles.append(sb)

            # lhs m-tile prep: quantize + xbar-transpose -> (qT, s_l)
            def prep_mtile(mi):
                lt = liop.tile([128, k], f32, tag="lt")
                nc.sync.dma_start(lt[:], lhs_d[ts(mi, 128), :])
                am = lstatp.tile([128, 1], f32, tag="am")
                nc.vector.tensor_reduce(
                    am[:],
                    lt[:],
                    axis=mybir.AxisListType.X,
                    op=vmax,
                    apply_absolute_value=True,
                )
                inv_l = lstatp.tile([128, 1], f32, tag="invl")
                nc.vector.reciprocal(inv_l[:], am[:])
                nc.vector.tensor_scalar_mul(inv_l[:], inv_l[:], 127.0)
                s_l = lstatp.tile([128, 1], f32, tag="sl")
                nc.vector.tensor_scalar_mul(s_l[:], am[:], 1.0 / 127.0)
                # scalar engine: in-place lt = lt*inv_l + MAGIC (rounds to int)
                nc.scalar.activation(
                    lt[:], lt[:], mybir.ActivationFunctionType.Copy,
                    bias=MAGIC, scale=inv_l[:],
                )
                qb = lqbp.tile([128, k], bf16, tag="qb")
                nc.scalar.activation(
                    qb[:], lt[:], mybir.ActivationFunctionType.Copy, bias=-MAGIC
                )
                qT = lqtp.tile([128, k], bf16, tag="qT")
                # one xbar-transpose DMA does all nk 128x128 block transposes:
                # out[p, b, f] = qb[f, b*128 + p]
                nc.sync.dma_start_transpose(
                    qT[:].rearrange("p (b f) -> p b f", f=128), qb[:]
                )
                return qT, s_l

            # rhs groups first: the per-column amax waits on the LAST group,
            # so rhs gets DMA priority; one lhs tile sneaks in near the tail.
            prepped = {}
            for g in range(ng):
                rhs_group(g)
                if g == ng - 4:
                    prepped[0] = prep_mtile(0)

            # ---------- fold max/min -> amax, allreduce, scales ------------
            t0 = rstatp.tile([128, n_loc], bf16, tag="t0")
            nc.vector.tensor_tensor(
                t0[:], accA[:, 0:n_loc], accA[:, n_loc : 2 * n_loc], op=vmax
            )
            t1 = rstatp.tile([128, n_loc], bf16, tag="t1")
            nc.vector.tensor_tensor(
                t1[:], accB[:, 0:n_loc], accB[:, n_loc : 2 * n_loc], op=vmin
            )
            accm = rstatp.tile([128, n_loc], f32, tag="accm")
            # accm = max(t1 * -1, t0)
            nc.vector.scalar_tensor_tensor(
                accm[:], t1[:], -1.0, t0[:], op0=mult, op1=vmax
            )
            amax_r = rstatp.tile([128, n_loc], f32, tag="amax_r")
            nc.gpsimd.partition_all_reduce(
                amax_r[:], accm[:], channels=128, reduce_op=bass_isa.ReduceOp.absmax
            )
            inv_r = rstatp.tile([128, n_loc], f32, tag="inv_r")
            nc.vector.reciprocal_approx_fast(inv_r[:], amax_r[:])
            inv_rb = rstatp.tile([128, n_loc], bf16, tag="inv_rb")
            nc.vector.tensor_scalar_mul(inv_rb[:], inv_r[:], 127.0)
            s_r = rstatp.tile([128, n_loc], f32, tag="s_r")
            nc.vector.tensor_scalar_mul(s_r[:], amax_r[:], 1.0 / 127.0)
            inv_rb2 = (
                inv_rb[:]
                .rearrange("p (o n) -> p o n", o=1)
                .broadcast_to((128, 2, n_loc))
            )

            # ---------- rhs pass 2 (SBUF only): quantize sb in place -------
            # ru = rne(sb * inv) via int16 convert; copy back as bf16.
            for g in range(ng):
                sb = sb_tiles[g]
                ru = rtmpp.tile([128, 2 * n_loc], i16, tag="ru")
                nc.vector.tensor_tensor(
                    ru[:].rearrange("p (o n) -> p o n", o=2),
                    sb[:].rearrange("p (o n) -> p o n", o=2),
                    inv_rb2,
                    op=mult,
                )
                nc.vector.tensor_scalar_mul(
                    sb[:, 0:n_loc], ru[:, 0:n_loc], 1.0
                )
                nc.vector.tensor_scalar_mul(
                    sb[:, n_loc : 2 * n_loc], ru[:, n_loc : 2 * n_loc], 1.0
                )

            def qr_ap(kk):  # quantized rhs k-tile kk as [128, n_loc] bf16
                return sb_tiles[kk // 2][:, (kk % 2) * n_loc : (kk % 2 + 1) * n_loc]

            def mm_mtile(mi, qT, s_l):
                for p in range(npan):
                    po = poutp.tile([128, panel], f32, tag="po")
                    for kk in range(nk):
                        nc.tensor.matmul(
                            po[:],
                            qT[:, ts(kk, 128)],
                            qr_ap(kk)[:, ts(p, panel)],
                            start=(kk == 0),
                            stop=(kk == nk - 1),
                        )
                    eo = eop.tile([128, panel], f32, tag="eo")
                    nc.vector.scalar_tensor_tensor(
                        eo[:], po[:], s_l[:], s_r[:, ts(p, panel)], op0=mult, op1=mult
                    )
                    nc.scalar.dma_start(out_d[ts(mi, 128), ts(p, panel)], eo[:])

            # ---------- m-tile loop: matmuls + epilogue, prepping ahead ----
            n_pre = 3
            for mi in range(1, n_pre):
                prepped[mi] = prep_mtile(mi)
            for mi in range(nm):
                if mi not in prepped:
                    prepped[mi] = prep_mtile(mi)
                qT, s_l = prepped.pop(mi)
                nxt = mi + n_pre
                if nxt < nm and nxt not in prepped:
                    prepped[nxt] = prep_mtile(nxt)
                mm_mtile(mi, qT, s_l)

    nc.compile()
    return nc


def run_shards(nc, lhs_shards, rhs_shards, trace=False, **kw):
    in_maps = [
        {"lhs": np.ascontiguousarray(l), "rhs": np.ascontiguousarray(r)}
        for l, r in zip(lhs_shards, rhs_shards)
    ]
    return run_bass_kernel_spmd(
        nc, in_maps, core_ids=list(range(len(in_maps))), trace=trace, **kw
    )


_NC_CACHE = {}


def get_full_nc():
    if "nc" not in _NC_CACHE:
        _NC_CACHE["nc"] = build_nc()
    return _NC_CACHE["nc"]


def kernel(lhs, rhs):
    lhs = np.ascontiguousarray(np.asarray(lhs, dtype=np.float32))
    rhs = np.ascontiguousarray(np.asarray(rhs, dtype=np.float32))
    assert lhs.shape == (B, M, K) and rhs.shape == (K, N)
    nc = get_full_nc()
    lhs_shards, rhs_shards = [], []
    for c in range(8):
        pi, qi = c // GRID_N, c % GRID_N
        lhs_shards.append(lhs[pi])
        rhs_shards.append(rhs[:, qi * N_LOC : (qi + 1) * N_LOC])
    res = run_shards(nc, lhs_shards, rhs_shards)
    out = np.empty((B, M, N), np.float32)
    for c in range(8):
        pi, qi = c // GRID_N, c % GRID_N
        out[pi, :, qi * N_LOC : (qi + 1) * N_LOC] = res.results[c]["out"]
    return out


if __name__ == "__main__":
    rng = np.random.default_rng(0)
    lhs = rng.standard_normal((B, M, K), dtype=np.float32)
    rhs = rng.standard_normal((K, N), dtype=np.float32)
    out = kernel(lhs=lhs, rhs=rhs)
    print("kernel output:", out.shape, out.dtype)


# revision 7
# speedup vs baseline: 1.0404x; 1.0074x over previous
"""AQT int8 symmetric-quantized dot_general (bmk,kn->bmn) on 8 TRN2 NeuronCores.

Problem: lhs [2, 4096, 4096] f32, rhs [4096, 4096] f32.
  q_l, s_l = absmax-int8-quantize(lhs, axis=K)   (per-row scales)
  q_r, s_r = absmax-int8-quantize(rhs, axis=K)   (per-col scales)
  out = (q_l @ q_r) * s_l * s_r                  [2, 4096, 4096] f32

Sharding: 2 (batch) x 4 (N columns) grid over 8 cores; K replicated.
Each core computes an independent [4096, 1024] output block - no collectives.

Per-core kernel (Tile framework), v4:
  - rhs single HBM pass: stream 16 groups of [128, 2x1024] f32; scalar engine
    keeps a persistent SIGNED bf16 copy (sb); DVE runs max and min
    accumulators (both bf16 2x mode; amax folds as max(max, -min) later).
    No ABS pass, no second HBM read of rhs.
  - gpsimd does ONLY memset + partition_all_reduce (mixing dma_start onto
    the gpsimd queue forces an ~11us library reload before the allreduce).
  - rhs quantize from SBUF: ru_int16 = rne(sb * inv) (DVE 2x, RNE convert),
    copied back into sb as bf16 - sb becomes q_r in place.
  - lhs per m-tile: DVE amax reduce; quantize multiply on the SCALAR engine
    (act(lt*inv_l + MAGIC) in place, then act(lt - MAGIC) -> bf16); one xbar
    DMA-transpose (Sync queue) puts K on partitions. The first two m-tiles'
    amax reduces are emitted mid-pass-1 where the DVE has slack; m2's fills
    the allreduce window; m3/m4's land after the pass-2 production ops.
  - catch-up phase: the first 2 m-tiles' matmuls are emitted kk-MAJOR so the
    PE consumes each quantized rhs k-pair the moment DVE produces it (panel-
    major emission would head-of-line block the PE FIFO on the last k-tile).
  - remaining m-tiles panel-major, prepping 3-4 ahead; epilogue
    (psum * s_l) * s_r in one DVE op; DMA out f32.
"""

import numpy as np

import concourse.bass as bass
import concourse.mybir as mybir
import concourse.tile as tile
from concourse import bacc, bass_isa
from concourse.bass import ts
from concourse.bass_utils import run_bass_kernel_spmd

MAGIC = 12582912.0  # 1.5 * 2**23: fp32 add => round-half-even to integer

B, M, K, N = 2, 4096, 4096, 4096
GRID_B, GRID_N = 2, 4  # 8 cores
M_LOC, N_LOC = M, N // GRID_N


def build_nc(m_loc=M_LOC, k=K, n_loc=N_LOC, panel=512):
    f32, bf16, i16 = mybir.dt.float32, mybir.dt.bfloat16, mybir.dt.int16
    mult, add = mybir.AluOpType.mult, mybir.AluOpType.add
    vmax, vmin = mybir.AluOpType.max, mybir.AluOpType.min
    nk, nm, npan = k // 128, m_loc // 128, n_loc // panel
    ng = nk // 2  # rhs DMA groups of 2 k-tiles
    n_catch = 2  # m-tiles consumed kk-major during rhs quantize production
    nc = bacc.Bacc("TRN2", target_bir_lowering=False, debug=False)
    lhs_d = nc.dram_tensor("lhs", [m_loc, k], f32, kind="ExternalInput")
    rhs_d = nc.dram_tensor("rhs", [k, n_loc], f32, kind="ExternalInput")
    out_d = nc.dram_tensor("out", [m_loc, n_loc], f32, kind="ExternalOutput")

    with tile.TileContext(nc) as tc:
        with (
            tc.tile_pool(name="rstat", bufs=1) as rstatp,
            tc.tile_pool(name="rio", bufs=3) as riop,
            tc.tile_pool(name="sb", bufs=1) as sbp,
            tc.tile_pool(name="rtmp", bufs=1) as rtmpp,
            tc.tile_pool(name="lio", bufs=3) as liop,
            tc.tile_pool(name="lqb", bufs=2) as lqbp,
            tc.tile_pool(name="lqt", bufs=3) as lqtp,
            tc.tile_pool(name="lstat", bufs=8) as lstatp,
            tc.tile_pool(name="eo", bufs=2) as eop,
            tc.tile_pool(name="pout", bufs=6, space="PSUM") as poutp,
        ):
            # ---------- rhs pass 1: stream + signed bf16 copy + max/min ----
            accA = rstatp.tile([128, 2 * n_loc], bf16, tag="accA")
            accB = rstatp.tile([128, 2 * n_loc], bf16, tag="accB")
            nc.gpsimd.memset(accA[:], 0.0)
            nc.gpsimd.memset(accB[:], 0.0)

            sb_tiles = []

            def rhs_group(g):
                rt = riop.tile([128, 2 * n_loc], f32, tag="rt")
                nc.sync.dma_start(
                    rt[:].rearrange("p (t n) -> p t n", t=2),
                    rhs_d[ts(g, 256), :].rearrange("(t p) n -> p t n", p=128),
                )
                sb = sbp.tile([128, 2 * n_loc], bf16, tag=f"sb{g}")
                nc.scalar.copy(sb[:], rt[:])
                nc.vector.tensor_tensor(accA[:], accA[:], sb[:], op=vmax)
                nc.vector.tensor_tensor(accB[:], accB[:], sb[:], op=vmin)
                sb_tiles.append(sb)

            # lhs m-tile prep, split into load (DMA) and compute phases so
            # the DVE amax can be placed where that engine has slack.
            lt_tiles = {}

            def prep_load(mi):
                lt = liop.tile([128, k], f32, tag="lt")
                nc.sync.dma_start(lt[:], lhs_d[ts(mi, 128), :])
                lt_tiles[mi] = lt

            def prep_compute(mi):
                lt = lt_tiles.pop(mi)
                am = lstatp.tile([128, 1], f32, tag="am")
                nc.vector.tensor_reduce(
                    am[:],
                    lt[:],
                    axis=mybir.AxisListType.X,
                    op=vmax,
                    apply_absolute_value=True,
                )
                inv_l = lstatp.tile([128, 1], f32, tag="invl")
                nc.vector.reciprocal(inv_l[:], am[:])
                nc.vector.tensor_scalar_mul(inv_l[:], inv_l[:], 127.0)
                s_l = lstatp.tile([128, 1], f32, tag="sl")
                nc.vector.tensor_scalar_mul(s_l[:], am[:], 1.0 / 127.0)
                # scalar engine: in-place lt = lt*inv_l + MAGIC (rounds to int)
                nc.scalar.activation(
                    lt[:], lt[:], mybir.ActivationFunctionType.Copy,
                    bias=MAGIC, scale=inv_l[:],
                )
                qb = lqbp.tile([128, k], bf16, tag="qb")
                nc.scalar.activation(
                    qb[:], lt[:], mybir.ActivationFunctionType.Copy, bias=-MAGIC
                )
                qT = lqtp.tile([128, k], bf16, tag="qT")
                # one xbar-transpose DMA does all nk 128x128 block transposes:
                # out[p, b, f] = qb[f, b*128 + p]
                nc.sync.dma_start_transpose(
                    qT[:].rearrange("p (b f) -> p b f", f=128), qb[:]
                )
                return qT, s_l

            # rhs groups stream first (per-column amax gates on the LAST
            # group). m0/m1 loads + amax interleave where DVE has slack.
            prepped = {}
            for g in range(ng):
                rhs_group(g)
                if g == 3:
                    prep_load(0)
                elif g == 6:
                    prepped[0] = prep_compute(0)
                    prep_load(1)
                elif g == 10:
                    prepped[1] = prep_compute(1)
                    prep_load(2)
            prep_load(3)

            # ---------- fold halves -> amax, allreduce, scales -------------
            # in-place: accA[:, :n] = max(halves), accB[:, :n] = min(halves)
            nc.vector.tensor_tensor(
                accA[:, 0:n_loc], accA[:, 0:n_loc], accA[:, n_loc : 2 * n_loc],
                op=vmax,
            )
            nc.vector.tensor_tensor(
                accB[:, 0:n_loc], accB[:, 0:n_loc], accB[:, n_loc : 2 * n_loc],
                op=vmin,
            )
            accm = rstatp.tile([128, n_loc], f32, tag="accm")
            # accm = max(accB * -1, accA)
            nc.vector.scalar_tensor_tensor(
                accm[:], accB[:, 0:n_loc], -1.0, accA[:, 0:n_loc],
                op0=mult, op1=vmax,
            )
            amax_r = rstatp.tile([128, n_loc], f32, tag="amax_r")
            nc.gpsimd.partition_all_reduce(
                amax_r[:], accm[:], channels=128, reduce_op=bass_isa.ReduceOp.absmax
            )
            # m2's amax fills the DVE while gpsimd runs the allreduce
            prepped[2] = prep_compute(2)
            inv_r = rstatp.tile([128, n_loc], f32, tag="accm")  # reuse slot
            nc.vector.reciprocal_approx_fast(inv_r[:], amax_r[:])
            inv_rb = rstatp.tile([128, n_loc], bf16, tag="inv_rb")
            nc.vector.tensor_scalar_mul(inv_rb[:], inv_r[:], 127.0)
            s_r = rstatp.tile([128, n_loc], f32, tag="s_r")
            nc.vector.tensor_scalar_mul(s_r[:], amax_r[:], 1.0 / 127.0)
            inv_rb2 = (
                inv_rb[:]
                .rearrange("p (o n) -> p o n", o=1)
                .broadcast_to((128, 2, n_loc))
            )

            # ---------- rhs pass 2 (SBUF only): quantize sb in place -------
            # ru = rne(sb * inv) via int16 convert; copy back as bf16.
            for g in range(ng):
                sb = sb_tiles[g]
                ru = rtmpp.tile([128, 2 * n_loc], i16, tag="ru")
                nc.vector.tensor_tensor(
                    ru[:].rearrange("p (o n) -> p o n", o=2),
                    sb[:].rearrange("p (o n) -> p o n", o=2),
                    inv_rb2,
                    op=mult,
                )
                nc.vector.tensor_scalar_mul(sb[:, 0:n_loc], ru[:, 0:n_loc], 1.0)
                nc.vector.tensor_scalar_mul(
                    sb[:, n_loc : 2 * n_loc], ru[:, n_loc : 2 * n_loc], 1.0
                )

            def qr_ap(kk):  # quantized rhs k-tile kk as [128, n_loc] bf16
                return sb_tiles[kk // 2][:, (kk % 2) * n_loc : (kk % 2 + 1) * n_loc]

            def epilogue(mi, p, po, s_l):
                eo = eop.tile([128, panel], f32, tag="eo")
                nc.vector.scalar_tensor_tensor(
                    eo[:], po[:], s_l[:], s_r[:, ts(p, panel)], op0=mult, op1=mult
                )
                nc.scalar.dma_start(out_d[ts(mi, 128), ts(p, panel)], eo[:])

            def mm_mtile(mi, qT, s_l):
                for p in range(npan):
                    po = poutp.tile([128, panel], f32, tag="po")
                    for kk in range(nk):
                        nc.tensor.matmul(
                            po[:],
                            qT[:, ts(kk, 128)],
                            qr_ap(kk)[:, ts(p, panel)],
                            start=(kk == 0),
                            stop=(kk == nk - 1),
                        )
                    epilogue(mi, p, po, s_l)

            # ---------- catch-up: m-tiles 0..n_catch-1 kk-major ------------
            catch_po = {}
            for m in range(n_catch):
                for p in range(npan):
                    po_c = poutp.tile([128, panel], f32, tag="po")
                    catch_po[(m, p)] = po_c
            # m3/m4 amax reduces land on DVE after the pass-2 production ops
            prep_load(4)
            prepped[3] = prep_compute(3)
            prepped[4] = prep_compute(4)
            for kk in range(nk):
                for m in range(n_catch):
                    qT, _ = prepped[m]
                    for p in range(npan):
                        nc.tensor.matmul(
                            catch_po[(m, p)][:],
                            qT[:, ts(kk, 128)],
                            qr_ap(kk)[:, ts(p, panel)],
                            start=(kk == 0),
                            stop=(kk == nk - 1),
                        )
            for m in range(n_catch):
                _, s_l = prepped.pop(m)
                for p in range(npan):
                    epilogue(m, p, catch_po[(m, p)], s_l)

            # ---------- steady m-tile loop, loads 4 / computes 3 ahead -----
            for mi in range(n_catch, nm):
                for j in range(mi + 1, min(mi + 5, nm)):
                    if j not in lt_tiles and j not in prepped:
                        prep_load(j)
                for j in range(mi + 1, min(mi + 4, nm)):
                    if j in lt_tiles and j not in prepped:
                        prepped[j] = prep_compute(j)
                if mi not in prepped:
                    prepped[mi] = prep_compute(mi)
                qT, s_l = prepped.pop(mi)
                mm_mtile(mi, qT, s_l)

    nc.compile()
    return nc


def run_shards(nc, lhs_shards, rhs_shards, trace=False, **kw):
    in_maps = [
        {"lhs": np.ascontiguousarray(l), "rhs": np.ascontiguousarray(r)}
        for l, r in zip(lhs_shards, rhs_shards)
    ]
    return run_bass_kernel_spmd(
        nc, in_maps, core_ids=list(range(len(in_maps))), trace=trace, **kw
    )


_NC_CACHE = {}


def get_full_nc():
    if "nc" not in _NC_CACHE:
        _NC_CACHE["nc"] = build_nc()
    return _NC_CACHE["nc"]


def kernel(lhs, rhs):
    lhs = np.ascontiguousarray(np.asarray(lhs, dtype=np.float32))
    rhs = np.ascontiguousarray(np.asarray(rhs, dtype=np.float32))
    assert lhs.shape == (B, M, K) and rhs.shape == (K, N)
    nc = get_full_nc()
    lhs_shards, rhs_shards = [], []
    for c in range(8):
        pi, qi = c // GRID_N, c % GRID_N
        lhs_shards.append(lhs[pi])
        rhs_shards.append(rhs[:, qi * N_LOC : (qi + 1) * N_LOC])
    res = run_shards(nc, lhs_shards, rhs_shards)
    out = np.empty((B, M, N), np.float32)
    for c in range(8):
        pi, qi = c // GRID_N, c % GRID_N
        out[pi, :, qi * N_LOC : (qi + 1) * N_LOC] = res.results[c]["out"]
    return out


if __name__ == "__main__":
    rng = np.random.default_rng(0)
    lhs = rng.standard_normal((B, M, K), dtype=np.float32)
    rhs = rng.standard_normal((K, N), dtype=np.float32)
    out = kernel(lhs=lhs, rhs=rhs)
    print("kernel output:", out.shape, out.dtype)
